# revision 1
# baseline (speedup 1.0000x reference)
"""Multi-head attention TRN2 kernel, head-parallel across 8 NeuronCores.

Per core c (= head h=c), all matmuls in float32r (11-bit mantissa, full PE
rate at N=512), keys-on-partitions score layout, with both outer
projections algebraically fused into the K / V projections:

  scores = q Wq (k Wk)^T = q G k^T          G = Wq Wk^T   (host)
  out    = attn (v Wv) Wo = attn (v U)      U = Wv Wo_h   (host)

so the device only computes, per core:

  K2T[d,t] = A k^T   with A = G^T = Wk Wq^T  (lhsT = A nat, rhs = kT)
  V2[t,o]  = v U                             (lhsT = vT,  rhs = U)
  scoresT[t,s] = K2 q^T                      (lhsT = K2T, rhs = qT chunk)
  E = exp(scoresT*scale + Madd + wbias[t])   (DVE mask-add, ACT exp)
  rowsum broadcast via ones[128,128] lhsT matmuls over E tiles
  outT[o,s] = V2^T E / rowsum                (lhsT = V2, rhs = E)

Host: transposes q/k/v, pre-rounds f32r inputs (RNE drop-12, bit-exact vs
HW cast), builds additive bf16 mask (0 / -1e9) in [t,s] orientation,
folds all biases exactly (bk drops under softmax; bq -> per-key exp
bias; bv,bo -> final add), sums per-head partial outputs and transposes
the [dout, s] device layout back to [b, s, dout].
"""
import sys
import numpy as np

sys.path.insert(0, "/opt/trn_rl_repo")

H, D, B, S = 8, 512, 2, 2048
P = 128
NE = D // P            # 4 feature tiles
NT = S // P            # 16 key tiles per batch
CH = 512               # query/key chunk width
NCH = S // CH          # 4 chunks per batch
SCALE = 1.0 / np.sqrt(np.float32(D))

_CACHE = {}


def _f32r_round(x):
    """Bit-exact host emulation of HW fp32->f32r cast (RNE, drop 12 bits)."""
    u = np.ascontiguousarray(x, np.float32).view(np.uint32).astype(np.uint64)
    half = np.uint64(1 << 11)
    lsb = (u >> np.uint64(12)) & np.uint64(1)
    u2 = (u + half - np.uint64(1) + lsb) >> np.uint64(12) << np.uint64(12)
    return u2.astype(np.uint32).view(np.float32).reshape(x.shape)


def _build():
    from contextlib import ExitStack
    from concourse import bass, bacc, tile

    mybir = bass.mybir
    dt = mybir.dt
    AF = mybir.ActivationFunctionType
    ALU = mybir.AluOpType

    nc = bacc.Bacc("TRN2", target_bir_lowering=False, debug=False)

    qT_d = nc.dram_tensor("qT", [D, B * S], dt.float32r, kind="ExternalInput")
    kT_d = nc.dram_tensor("kT", [D, B * S], dt.float32r, kind="ExternalInput")
    vT_d = nc.dram_tensor("vT", [D, B * S], dt.float32r, kind="ExternalInput")
    mT_d = nc.dram_tensor("mT", [S, S], dt.bfloat16, kind="ExternalInput")
    ka_d = nc.dram_tensor("ka", [D, D], dt.float32r, kind="ExternalInput")  # Wk Wq^T
    vu_d = nc.dram_tensor("vu", [D, D], dt.float32r, kind="ExternalInput")  # Wv Wo_h
    wb_d = nc.dram_tensor("wb", [P, B * NT], dt.float32, kind="ExternalInput")
    out_d = nc.dram_tensor("out", [D, B * S], dt.float32, kind="ExternalOutput")
    rs_d = nc.dram_tensor("rs", [P, B * S], dt.float32, kind="ExternalOutput")

    def dtiles(ap_2d):
        return ap_2d.rearrange("(a p) c -> p a c", p=P)

    with tile.TileContext(nc) as tc:
        with ExitStack() as ctx:
            wpool = ctx.enter_context(tc.tile_pool(name="w", bufs=1))
            kvpool = ctx.enter_context(tc.tile_pool(name="kv", bufs=1))
            xin = ctx.enter_context(tc.tile_pool(name="xin", bufs=4))
            epool = ctx.enter_context(tc.tile_pool(name="e", bufs=1))
            mpool = ctx.enter_context(tc.tile_pool(name="m", bufs=2))
            tpool = ctx.enter_context(tc.tile_pool(name="tmp", bufs=3))
            rpool = ctx.enter_context(tc.tile_pool(name="r", bufs=1))
            opool = ctx.enter_context(tc.tile_pool(name="o", bufs=3))
            psA = ctx.enter_context(tc.tile_pool(name="psA", bufs=4, space="PSUM"))
            psO = ctx.enter_context(tc.tile_pool(name="psO", bufs=4, space="PSUM"))

            ka = wpool.tile([P, NE, D], dt.float32r)
            vu = wpool.tile([P, NE, D], dt.float32r)
            nc.sync.dma_start(ka[:], dtiles(ka_d.ap()))
            wb = wpool.tile([P, B * NT], dt.float32)


            K2T = kvpool.tile([P, NE, S], dt.float32r, tag="K2T")
            V2 = kvpool.tile([P, NT, D], dt.float32r, tag="V2")

            qTt = dtiles(qT_d.ap())
            kTt = dtiles(kT_d.ap())
            vTt = dtiles(vT_d.ap())
            mTt = mT_d.ap().rearrange("(a p) c -> p a c", p=P)  # [128, NT, S]

            for b in range(B):
                # ---- stage A: K2^T first (scores-critical), then V2 ----
                qin0 = None
                for tc4 in range(NCH):
                    col0 = b * S + tc4 * CH
                    kin = xin.tile([P, NE, CH], dt.float32r, tag="xin")
                    nc.sync.dma_start(kin[:], kTt[:, :, col0:col0 + CH])
                    if tc4 == 2:
                        qin0 = xin.tile([P, NE, CH], dt.float32r, tag="xin")
                        nc.sync.dma_start(qin0[:], qTt[:, :, b * S:b * S + CH])
                    for et in range(NE):
                        ps = psA.tile([P, CH], dt.float32, tag="ps")
                        for kd in range(NE):
                            nc.tensor.matmul(
                                ps[:], ka[:, kd, et * P:(et + 1) * P], kin[:, kd, :],
                                start=(kd == 0), stop=(kd == NE - 1))
                        nc.scalar.copy(K2T[:, et, tc4 * CH:(tc4 + 1) * CH], ps[:])
                if b == 0:
                    nc.sync.dma_start(vu[:], dtiles(vu_d.ap()))
                    nc.sync.dma_start(wb[:], wb_d[:])
                for tc4 in range(NCH):
                    col0 = b * S + tc4 * CH
                    vin = xin.tile([P, NE, CH], dt.float32r, tag="xin")
                    nc.sync.dma_start(vin[:], vTt[:, :, col0:col0 + CH])
                    for ts in range(CH // P):
                        ps = psA.tile([P, D], dt.float32, tag="ps")
                        for kd in range(NE):
                            nc.tensor.matmul(
                                ps[:], vin[:, kd, ts * P:(ts + 1) * P], vu[:, kd, :],
                                start=(kd == 0), stop=(kd == NE - 1))
                        nc.scalar.copy(V2[:, tc4 * (CH // P) + ts, :], ps[:])

                # ---- stage B: per query-chunk attention ----
                for c in range(NCH):
                    col0 = b * S + c * CH
                    if c == 0:
                        qin = qin0
                    else:
                        qin = xin.tile([P, NE, CH], dt.float32r, tag="xin")
                        nc.sync.dma_start(qin[:], qTt[:, :, col0:col0 + CH])
                    mt = mpool.tile([P, NT, CH], dt.bfloat16)
                    nc.gpsimd.dma_start(mt[:], mTt[:, :, c * CH:(c + 1) * CH])

                    E = epool.tile([P, NT, CH], dt.float32r)
                    for tt in range(NT):
                        ps = psA.tile([P, CH], dt.float32, tag="ps")
                        for et in range(NE):
                            nc.tensor.matmul(
                                ps[:], K2T[:, et, tt * P:(tt + 1) * P], qin[:, et, :],
                                start=(et == 0), stop=(et == NE - 1))
                        tmp = tpool.tile([P, CH], dt.float32)
                        nc.vector.scalar_tensor_tensor(
                            tmp[:], ps[:], float(SCALE), mt[:, tt, :],
                            op0=ALU.mult, op1=ALU.add)
                        nc.scalar.activation(
                            E[:, tt, :], tmp[:], AF.Exp,
                            bias=wb[:, b * NT + tt: b * NT + tt + 1], scale=1.0)

                    Ef = E[:].bitcast(dt.float32)
                    red = rpool.tile([P, NT // 2, CH], dt.float32, tag="red")
                    nc.vector.tensor_add(red[:], Ef[:, 0:8, :], Ef[:, 8:16, :])
                    nc.vector.tensor_add(red[:, 0:4, :], red[:, 0:4, :], red[:, 4:8, :])
                    nc.vector.tensor_add(red[:, 0:2, :], red[:, 0:2, :], red[:, 2:4, :])
                    accr = rpool.tile([P, CH], dt.float32, tag="accr")
                    nc.vector.tensor_add(accr[:], red[:, 0, :], red[:, 1, :])
                    nc.gpsimd.dma_start(rs_d[:, col0:col0 + CH], accr[:])

                    pso = [psO.tile([P, CH], dt.float32, tag="pso", name=f"pso{i}") for i in range(NE)]
                    for tt in range(NT):
                        for os_ in range(NE):
                            nc.tensor.matmul(
                                pso[os_][:], V2[:, tt, os_ * P:(os_ + 1) * P],
                                E[:, tt, :],
                                start=(tt == 0), stop=(tt == NT - 1))
                    for os_ in range(NE):
                        ot = opool.tile([P, CH], dt.float32)
                        nc.scalar.copy(ot[:], pso[os_][:])
                        r0 = os_ * P
                        nc.gpsimd.dma_start(out_d[r0:r0 + P, col0:col0 + CH], ot[:])

    nc.compile()
    return nc


def kernel(q, k, v, mask, Wq, bq, Wk, bk, Wv, bv, Wo, bo):
    from concourse.bass_utils import run_bass_kernel_spmd
    import ml_dtypes

    q = np.asarray(q, np.float32)
    k = np.asarray(k, np.float32)
    v = np.asarray(v, np.float32)
    mask = np.asarray(mask)
    Wq = np.asarray(Wq, np.float32)
    Wk = np.asarray(Wk, np.float32)
    Wv = np.asarray(Wv, np.float32)
    Wo = np.asarray(Wo, np.float32)
    bq = np.asarray(bq, np.float32)
    bk = np.asarray(bk, np.float32)
    bv = np.asarray(bv, np.float32)
    bo = np.asarray(bo, np.float32)

    qT = _f32r_round(q.transpose(2, 0, 1).reshape(D, B * S))
    kT = _f32r_round(k.transpose(2, 0, 1).reshape(D, B * S))
    vT = _f32r_round(v.transpose(2, 0, 1).reshape(D, B * S))
    mT = np.where(mask.T == 1, np.float32(-1e9), np.float32(0.0)).astype(ml_dtypes.bfloat16)
    mT = np.ascontiguousarray(mT)

    kf = k.reshape(B * S, D)
    in_maps = []
    for h in range(H):
        Wq64 = Wq[h].astype(np.float64)
        Wk64 = Wk[h].astype(np.float64)
        Wv64 = Wv[h].astype(np.float64)
        Wo64 = Wo[h * D:(h + 1) * D, :].astype(np.float64)
        A = (Wk64 @ Wq64.T).astype(np.float32)       # lhsT for K2^T proj
        U = (Wv64 @ Wo64).astype(np.float32)         # rhs for V2 proj
        wvec = (kf @ (Wk[h] @ bq[h])) * SCALE        # per-key exp bias
        wb = np.ascontiguousarray(wvec.reshape(B * NT, P).T.astype(np.float32))
        in_maps.append({
            "qT": qT, "kT": kT, "vT": vT, "mT": mT,
            "ka": _f32r_round(A), "vu": _f32r_round(U), "wb": wb,
        })

    if "nc" not in _CACHE:
        _CACHE["nc"] = _build()
    nc = _CACHE["nc"]
    _CACHE["in_maps"] = in_maps

    res = run_bass_kernel_spmd(nc, in_maps, core_ids=list(range(H)))
    total = np.zeros((D, B * S), np.float64)
    for h in range(H):
        r = res.results[h]["rs"].sum(axis=0, dtype=np.float64)   # [B*S]
        total += res.results[h]["out"].astype(np.float64) / r[None, :]

    cvec = bo.astype(np.float64).copy()
    for h in range(H):
        cvec += bv[h].astype(np.float64) @ Wo[h * D:(h + 1) * D, :].astype(np.float64)
    total += cvec[:, None]
    return total.T.astype(np.float32).reshape(B, S, D)



# revision 3
# speedup vs baseline: 1.0826x; 1.0826x over previous
"""Multi-head attention TRN2 kernel, head-parallel across 8 NeuronCores.

Per core c (= head h=c), all matmuls in bf16 (full PE rate, half the DMA
and SBUF footprint of f32), keys-on-partitions score layout, with both
outer projections algebraically fused into the K / V projections:

  scores = q Wq (k Wk)^T = q G k^T          G = Wq Wk^T   (host)
  out    = attn (v Wv) Wo = attn (v U)      U = Wv Wo_h   (host)

so the device only computes, per core:

  K2T[e,t] = A k^T   with A = (Wk Wq^T)*scale  (lhsT = A nat, rhs = kT)
  V2[t,o]  = v U                               (lhsT = vT,  rhs = U)
  scoresT[t,s] = K2 q^T                        (lhsT = K2T, rhs = qT chunk)
  E = exp(scoresT + wbias[t]) * m01[t,s]       (ACT exp from PSUM, DVE mask)
  rowsum partials via DVE add tree over E tiles -> rs output
  outT[o,s] = V2^T E                           (lhsT = V2, rhs = E)

Scheduling: 12 warm-up matmuls on a memset tile keep the PE HAM clock
gate from running the first ~25us at 1.2 GHz; E / K2T / V2 / mask / q
are double-buffered so chunks pipeline with no PE idle; PSUM evacuation
is split between ACT (K2T) and DVE (V2, out) to keep both ~50% busy.

Host: transposes + bf16-casts q/k/v, builds {0,1} bf16 multiplicative
mask in [t,s] orientation, folds all biases exactly (bk drops under
softmax; bq -> per-key exp bias; bv,bo -> final add), sums per-head
partial outputs, divides by the gathered rowsums, and transposes the
[dout, s] device layout back to [b, s, dout].
"""
import sys
import numpy as np

sys.path.insert(0, "/opt/trn_rl_repo")

H, D, B, S = 8, 512, 2, 2048
P = 128
NE = D // P            # 4 feature tiles
NT = S // P            # 16 key tiles per batch
CH = 512               # query/key chunk width
NCH = S // CH          # 4 chunks per batch
SCALE = 1.0 / np.sqrt(np.float32(D))

_CACHE = {}


def _build():
    from contextlib import ExitStack
    from concourse import bass, bacc, tile

    mybir = bass.mybir
    dt = mybir.dt
    AF = mybir.ActivationFunctionType

    nc = bacc.Bacc("TRN2", target_bir_lowering=False, debug=False)

    qT_d = nc.dram_tensor("qT", [D, B * S], dt.bfloat16, kind="ExternalInput")
    kT_d = nc.dram_tensor("kT", [D, B * S], dt.bfloat16, kind="ExternalInput")
    vT_d = nc.dram_tensor("vT", [D, B * S], dt.bfloat16, kind="ExternalInput")
    mT_d = nc.dram_tensor("mT", [S, S], dt.bfloat16, kind="ExternalInput")
    ka_d = nc.dram_tensor("ka", [D, D], dt.bfloat16, kind="ExternalInput")  # (Wk Wq^T)*scale
    vu_d = nc.dram_tensor("vu", [D, D], dt.bfloat16, kind="ExternalInput")  # Wv Wo_h
    wb_d = nc.dram_tensor("wb", [P, B * NT], dt.float32, kind="ExternalInput")
    out_d = nc.dram_tensor("out", [D, B * S], dt.bfloat16, kind="ExternalOutput")
    rs_d = nc.dram_tensor("rs", [P, B * S], dt.float32, kind="ExternalOutput")

    def dtiles(ap_2d):
        return ap_2d.rearrange("(a p) c -> p a c", p=P)

    with tile.TileContext(nc) as tc:
        with ExitStack() as ctx:
            wpool = ctx.enter_context(tc.tile_pool(name="w", bufs=1))
            kvpool = ctx.enter_context(tc.tile_pool(name="kv", bufs=2))
            xin = ctx.enter_context(tc.tile_pool(name="xin", bufs=3))
            qpool = ctx.enter_context(tc.tile_pool(name="q", bufs=2))
            epool = ctx.enter_context(tc.tile_pool(name="e", bufs=2))
            mpool = ctx.enter_context(tc.tile_pool(name="m", bufs=2))
            rpool = ctx.enter_context(tc.tile_pool(name="r", bufs=1))
            opool = ctx.enter_context(tc.tile_pool(name="o", bufs=2))
            psA = ctx.enter_context(tc.tile_pool(name="psA", bufs=4, space="PSUM"))
            psO = ctx.enter_context(tc.tile_pool(name="psO", bufs=4, space="PSUM"))

            # --- PE warm-up: ~12 matmuls on a zeroed tile so the HAM
            # clock gate reaches 8/8 before the first real matmul. ---
            warm = wpool.tile([P, CH], dt.bfloat16)
            nc.gpsimd.memset(warm[:], 0.0)
            for i in range(12):
                pw = psO.tile([P, CH], dt.float32, tag="pso")
                nc.tensor.matmul(pw[:], warm[:, 0:P], warm[:], start=True, stop=True)

            ka = wpool.tile([P, NE, D], dt.bfloat16)
            vu = wpool.tile([P, NE, D], dt.bfloat16)
            wb = wpool.tile([P, B * NT], dt.float32)
            nc.sync.dma_start(ka[:], dtiles(ka_d.ap()))
            nc.sync.dma_start(vu[:], dtiles(vu_d.ap()))
            nc.sync.dma_start(wb[:], wb_d[:])

            qTt = dtiles(qT_d.ap())
            kTt = dtiles(kT_d.ap())
            vTt = dtiles(vT_d.ap())
            mTt = mT_d.ap().rearrange("(a p) c -> p a c", p=P)  # [128, NT, S]
            oTt = dtiles(out_d.ap())

            for b in range(B):
                # ---- stage A: K2^T first (scores-critical), then V2 ----
                K2T = kvpool.tile([P, NE, S], dt.bfloat16, tag="K2T")
                V2 = kvpool.tile([P, NT, D], dt.bfloat16, tag="V2")
                for tc4 in range(NCH):
                    col0 = b * S + tc4 * CH
                    kin = xin.tile([P, NE, CH], dt.bfloat16, tag="xin")
                    nc.sync.dma_start(kin[:], kTt[:, :, col0:col0 + CH])
                    for et in range(NE):
                        ps = psA.tile([P, CH], dt.float32, tag="ps")
                        for kd in range(NE):
                            nc.tensor.matmul(
                                ps[:], ka[:, kd, et * P:(et + 1) * P], kin[:, kd, :],
                                start=(kd == 0), stop=(kd == NE - 1))
                        nc.scalar.copy(K2T[:, et, tc4 * CH:(tc4 + 1) * CH], ps[:])
                for tc4 in range(NCH):
                    col0 = b * S + tc4 * CH
                    vin = xin.tile([P, NE, CH], dt.bfloat16, tag="xin")
                    nc.sync.dma_start(vin[:], vTt[:, :, col0:col0 + CH])
                    for ts in range(CH // P):
                        ps = psA.tile([P, D], dt.float32, tag="ps")
                        for kd in range(NE):
                            nc.tensor.matmul(
                                ps[:], vin[:, kd, ts * P:(ts + 1) * P], vu[:, kd, :],
                                start=(kd == 0), stop=(kd == NE - 1))
                        nc.vector.tensor_copy(V2[:, tc4 * (CH // P) + ts, :], ps[:])

                # ---- stage B: per query-chunk attention ----
                for c in range(NCH):
                    col0 = b * S + c * CH
                    qin = qpool.tile([P, NE, CH], dt.bfloat16, tag="qin")
                    nc.sync.dma_start(qin[:], qTt[:, :, col0:col0 + CH])
                    mt = mpool.tile([P, NT, CH], dt.bfloat16)
                    nc.gpsimd.dma_start(mt[:], mTt[:, :, c * CH:(c + 1) * CH])

                    E = epool.tile([P, NT, CH], dt.bfloat16)
                    for tt in range(NT):
                        ps = psA.tile([P, CH], dt.float32, tag="ps")
                        for et in range(NE):
                            nc.tensor.matmul(
                                ps[:], K2T[:, et, tt * P:(tt + 1) * P], qin[:, et, :],
                                start=(et == 0), stop=(et == NE - 1))
                        nc.scalar.activation(
                            E[:, tt, :], ps[:], AF.Exp,
                            bias=wb[:, b * NT + tt: b * NT + tt + 1], scale=1.0)
                        nc.vector.tensor_mul(E[:, tt, :], E[:, tt, :], mt[:, tt, :])

                    red = rpool.tile([P, NT // 2, CH], dt.float32, tag="red")
                    nc.vector.tensor_add(red[:], E[:, 0:8, :], E[:, 8:16, :])
                    nc.vector.tensor_add(red[:, 0:4, :], red[:, 0:4, :], red[:, 4:8, :])
                    nc.vector.tensor_add(red[:, 0:2, :], red[:, 0:2, :], red[:, 2:4, :])
                    accr = rpool.tile([P, CH], dt.float32, tag="accr")
                    nc.vector.tensor_add(accr[:], red[:, 0, :], red[:, 1, :])
                    nc.gpsimd.dma_start(rs_d[:, col0:col0 + CH], accr[:])

                    pso = [psO.tile([P, CH], dt.float32, tag="pso", name=f"pso{i}") for i in range(NE)]
                    for tt in range(NT):
                        for os_ in range(NE):
                            nc.tensor.matmul(
                                pso[os_][:], V2[:, tt, os_ * P:(os_ + 1) * P],
                                E[:, tt, :],
                                start=(tt == 0), stop=(tt == NT - 1))
                    ot = opool.tile([P, NE, CH], dt.bfloat16)
                    for os_ in range(NE):
                        nc.vector.tensor_copy(ot[:, os_, :], pso[os_][:])
                    nc.gpsimd.dma_start(oTt[:, :, col0:col0 + CH], ot[:])

    nc.compile()
    return nc


def kernel(q, k, v, mask, Wq, bq, Wk, bk, Wv, bv, Wo, bo):
    from concourse.bass_utils import run_bass_kernel_spmd
    import ml_dtypes

    q = np.asarray(q, np.float32)
    k = np.asarray(k, np.float32)
    v = np.asarray(v, np.float32)
    mask = np.asarray(mask)
    Wq = np.asarray(Wq, np.float32)
    Wk = np.asarray(Wk, np.float32)
    Wv = np.asarray(Wv, np.float32)
    Wo = np.asarray(Wo, np.float32)
    bq = np.asarray(bq, np.float32)
    bk = np.asarray(bk, np.float32)
    bv = np.asarray(bv, np.float32)
    bo = np.asarray(bo, np.float32)

    bf16 = ml_dtypes.bfloat16
    qT = np.ascontiguousarray(q.transpose(2, 0, 1).reshape(D, B * S)).astype(bf16)
    kT = np.ascontiguousarray(k.transpose(2, 0, 1).reshape(D, B * S)).astype(bf16)
    vT = np.ascontiguousarray(v.transpose(2, 0, 1).reshape(D, B * S)).astype(bf16)
    mT = np.where(mask.T == 1, np.float32(0.0), np.float32(1.0)).astype(bf16)
    mT = np.ascontiguousarray(mT)

    kf = k.reshape(B * S, D)
    in_maps = []
    for h in range(H):
        Wq64 = Wq[h].astype(np.float64)
        Wk64 = Wk[h].astype(np.float64)
        Wv64 = Wv[h].astype(np.float64)
        Wo64 = Wo[h * D:(h + 1) * D, :].astype(np.float64)
        A = (Wk64 @ Wq64.T * SCALE).astype(np.float32)  # lhsT for K2^T proj
        U = (Wv64 @ Wo64).astype(np.float32)            # rhs for V2 proj
        wvec = (kf @ (Wk[h] @ bq[h])) * SCALE           # per-key exp bias
        wb = np.ascontiguousarray(wvec.reshape(B * NT, P).T.astype(np.float32))
        in_maps.append({
            "qT": qT, "kT": kT, "vT": vT, "mT": mT,
            "ka": A.astype(bf16), "vu": U.astype(bf16), "wb": wb,
        })

    if "nc" not in _CACHE:
        _CACHE["nc"] = _build()
    nc = _CACHE["nc"]
    _CACHE["in_maps"] = in_maps

    res = run_bass_kernel_spmd(nc, in_maps, core_ids=list(range(H)))
    total = np.zeros((D, B * S), np.float64)
    for h in range(H):
        r = res.results[h]["rs"].sum(axis=0, dtype=np.float64)   # [B*S]
        total += res.results[h]["out"].astype(np.float64) / r[None, :]

    cvec = bo.astype(np.float64).copy()
    for h in range(H):
        cvec += bv[h].astype(np.float64) @ Wo[h * D:(h + 1) * D, :].astype(np.float64)
    total += cvec[:, None]
    return total.T.astype(np.float32).reshape(B, S, D)


# revision 6
# speedup vs baseline: 1.0834x; 1.0007x over previous
"""Multi-head attention TRN2 kernel, head-parallel across 8 NeuronCores.

Per core c (= head h=c), all matmuls in bf16 (full PE rate, half the DMA
and SBUF footprint of f32), keys-on-partitions score layout, with both
outer projections algebraically fused into the K / V projections:

  scores = q Wq (k Wk)^T = q G k^T          G = Wq Wk^T   (host)
  out    = attn (v Wv) Wo = attn (v U)      U = Wv Wo_h   (host)

so the device only computes, per core:

  K2T[e,t] = A k^T   with A = (Wk Wq^T)*scale  (lhsT = A nat, rhs = kT)
  V2[t,o]  = v U                               (lhsT = vT,  rhs = U)
  scoresT[t,s] = K2 q^T                        (lhsT = K2T, rhs = qT chunk)
  E = exp(scoresT + wbias[t]) * m01[t,s]       (ACT exp from PSUM, DVE mask)
  rowsum partials via DVE add tree over E tiles -> rs output
  outT[o,s] = V2^T E                           (lhsT = V2, rhs = E)

Scheduling: 12 dependency-free warm-up matmuls keep the PE HAM clock
gate from throttling the start; all DRAM tensors use partition-major
layouts so every DMA moves 4-16 KB contiguous lines; k/v stream in as
per-feature-plane transfers so the first projection matmul only waits
on 0.5 MB; both batches' K2T/V2 are built first, then stage B walks
(chunk, batch) pairs so each mask chunk is fetched once; E / K2T / V2 /
mask / q are double-buffered; PSUM evacuation is split between ACT
(K2T) and DVE (V2, out).

Host: bf16-casts and relayouts inputs, folds all biases exactly (bk
drops under softmax; bq -> per-key exp bias; bv,bo -> final add), sums
per-head partial outputs, divides by the gathered rowsums, and undoes
the partition-major output layout.
"""
import sys
import numpy as np

sys.path.insert(0, "/opt/trn_rl_repo")

H, D, B, S = 8, 512, 2, 2048
P = 128
NE = D // P            # 4 feature tiles
NT = S // P            # 16 key tiles per batch
CH = 512               # query/key chunk width
NCH = S // CH          # 4 chunks per batch
SCALE = 1.0 / np.sqrt(np.float32(D))

_CACHE = {}


def _build():
    from contextlib import ExitStack
    from concourse import bass, bacc, tile

    mybir = bass.mybir
    dt = mybir.dt
    AF = mybir.ActivationFunctionType

    nc = bacc.Bacc("TRN2", target_bir_lowering=False, debug=False)

    # All DRAM tensors are partition-major: [128, ...free...] with the
    # free axis laid out exactly as the SBUF tiles consume it, so DMA
    # lines are 4-16 KB contiguous.
    qT_d = nc.dram_tensor("qT", [P, B * NCH * NE * CH], dt.bfloat16, kind="ExternalInput")
    kT_d = nc.dram_tensor("kT", [P, NE * B * S], dt.bfloat16, kind="ExternalInput")
    vT_d = nc.dram_tensor("vT", [P, NE * B * S], dt.bfloat16, kind="ExternalInput")
    mT_d = nc.dram_tensor("mT", [P, NCH * NT * CH], dt.bfloat16, kind="ExternalInput")
    ka_d = nc.dram_tensor("ka", [P, NE * D], dt.bfloat16, kind="ExternalInput")  # (Wk Wq^T)*scale
    vu_d = nc.dram_tensor("vu", [P, NE * D], dt.bfloat16, kind="ExternalInput")  # Wv Wo_h
    wb_d = nc.dram_tensor("wb", [P, B * NT], dt.float32, kind="ExternalInput")
    out_d = nc.dram_tensor("out", [P, B * NCH * NE * CH], dt.bfloat16, kind="ExternalOutput")
    rs_d = nc.dram_tensor("rs", [P, B * S], dt.float32, kind="ExternalOutput")

    q4 = qT_d.ap().rearrange("p (k a c) -> p k a c", a=NE, c=CH)    # [128, B*NCH, NE, CH]
    k4 = kT_d.ap().rearrange("p (a b t) -> p a b t", a=NE, b=B)     # [128, NE, B, S]
    v4 = vT_d.ap().rearrange("p (a b t) -> p a b t", a=NE, b=B)
    m4 = mT_d.ap().rearrange("p (c a s) -> p c a s", c=NCH, a=NT)   # [128, NCH, NT, CH]
    ka4 = ka_d.ap().rearrange("p (a e) -> p a e", a=NE)             # [128, NE, D]
    vu4 = vu_d.ap().rearrange("p (a e) -> p a e", a=NE)
    o4 = out_d.ap().rearrange("p (k a c) -> p k a c", a=NE, c=CH)   # [128, B*NCH, NE, CH]

    with tile.TileContext(nc) as tc:
        with ExitStack() as ctx:
            wpool = ctx.enter_context(tc.tile_pool(name="w", bufs=1))
            kvpool = ctx.enter_context(tc.tile_pool(name="kv", bufs=2))
            xin = ctx.enter_context(tc.tile_pool(name="xin", bufs=2))
            qpool = ctx.enter_context(tc.tile_pool(name="q", bufs=2))
            epool = ctx.enter_context(tc.tile_pool(name="e", bufs=2))
            mpool = ctx.enter_context(tc.tile_pool(name="m", bufs=2))
            rpool = ctx.enter_context(tc.tile_pool(name="r", bufs=1))
            opool = ctx.enter_context(tc.tile_pool(name="o", bufs=1))
            psA = ctx.enter_context(tc.tile_pool(name="psA", bufs=4, space="PSUM"))
            psO = ctx.enter_context(tc.tile_pool(name="psO", bufs=4, space="PSUM"))

            # --- PE warm-up: matmuls on a zeroed tile so the HAM clock
            # gate reaches 8/8 before real work arrives. ---
            warm = wpool.tile([P, CH], dt.bfloat16)
            nc.vector.memset(warm[:], 0.0)
            for i in range(12):
                pw = psO.tile([P, CH], dt.float32, tag="pso")
                nc.tensor.matmul(pw[:], warm[:, 0:P], warm[:], start=True, stop=True)

            ka = wpool.tile([P, NE, D], dt.bfloat16)
            vu = wpool.tile([P, NE, D], dt.bfloat16)
            wb = wpool.tile([P, B * NT], dt.float32)
            nc.sync.dma_start(ka[:], ka4[:, :, :])

            K2Ts, V2s, kins, vins = {}, {}, {}, {}
            for b in range(B):
                kins[b] = xin.tile([P, NE, S], dt.bfloat16, tag="xin", name=f"kin{b}")
                for kd in range(NE):
                    nc.sync.dma_start(kins[b][:, kd, :], k4[:, kd, b, :])
            nc.sync.dma_start(vu[:], vu4[:, :, :])
            nc.sync.dma_start(wb[:], wb_d[:])
            for b in range(B):
                vins[b] = xin.tile([P, NE, S], dt.bfloat16, tag="xin", name=f"vin{b}")
                for kd in range(NE):
                    nc.sync.dma_start(vins[b][:, kd, :], v4[:, kd, b, :])

            # ---- stage A: K2^T (scores-critical) then V2, both batches ----
            for b in range(B):
                K2Ts[b] = kvpool.tile([P, NE, S], dt.bfloat16, tag="K2T", name=f"K2T{b}")
                kin = kins[b]
                for tc4 in range(NCH):
                    for et in range(NE):
                        ps = psA.tile([P, CH], dt.float32, tag="ps")
                        for kd in range(NE):
                            nc.tensor.matmul(
                                ps[:], ka[:, kd, et * P:(et + 1) * P],
                                kin[:, kd, tc4 * CH:(tc4 + 1) * CH],
                                start=(kd == 0), stop=(kd == NE - 1))
                        nc.scalar.copy(K2Ts[b][:, et, tc4 * CH:(tc4 + 1) * CH], ps[:])
            for b in range(B):
                V2s[b] = kvpool.tile([P, NT, D], dt.bfloat16, tag="V2", name=f"V2{b}")
                vin = vins[b]
                for tc4 in range(NCH):
                    for ts in range(CH // P):
                        ps = psA.tile([P, D], dt.float32, tag="ps")
                        for kd in range(NE):
                            nc.tensor.matmul(
                                ps[:], vin[:, kd, (tc4 * (CH // P) + ts) * P:(tc4 * (CH // P) + ts + 1) * P],
                                vu[:, kd, :],
                                start=(kd == 0), stop=(kd == NE - 1))
                        nc.vector.tensor_copy(V2s[b][:, tc4 * (CH // P) + ts, :], ps[:])

            # ---- stage B: per (chunk, batch) attention; mask loaded once per chunk ----
            for c in range(NCH):
                mt = mpool.tile([P, NT, CH], dt.bfloat16)
                nc.gpsimd.dma_start(mt[:], m4[:, c, :, :])
                for b in range(B):
                    blk = b * NCH + c
                    col0 = b * S + c * CH
                    K2T, V2 = K2Ts[b], V2s[b]
                    qin = qpool.tile([P, NE, CH], dt.bfloat16, tag="qin")
                    nc.sync.dma_start(qin[:], q4[:, blk, :, :])

                    E = epool.tile([P, NT, CH], dt.bfloat16)
                    for tt in range(NT):
                        ps = psA.tile([P, CH], dt.float32, tag="ps")
                        for et in range(NE):
                            nc.tensor.matmul(
                                ps[:], K2T[:, et, tt * P:(tt + 1) * P], qin[:, et, :],
                                start=(et == 0), stop=(et == NE - 1))
                        nc.scalar.activation(
                            E[:, tt, :], ps[:], AF.Exp,
                            bias=wb[:, b * NT + tt: b * NT + tt + 1], scale=1.0)
                        nc.vector.tensor_mul(E[:, tt, :], E[:, tt, :], mt[:, tt, :])

                    red = rpool.tile([P, NT // 2, CH], dt.float32, tag="red")
                    nc.vector.tensor_add(red[:], E[:, 0:8, :], E[:, 8:16, :])
                    nc.vector.tensor_add(red[:, 0:4, :], red[:, 0:4, :], red[:, 4:8, :])
                    nc.vector.tensor_add(red[:, 0:2, :], red[:, 0:2, :], red[:, 2:4, :])
                    accr = rpool.tile([P, CH], dt.float32, tag="accr")
                    nc.vector.tensor_add(accr[:], red[:, 0, :], red[:, 1, :])
                    nc.gpsimd.dma_start(rs_d[:, col0:col0 + CH], accr[:])

                    pso = [psO.tile([P, CH], dt.float32, tag="pso", name=f"pso{i}") for i in range(NE)]
                    for tt in range(NT):
                        for os_ in range(NE):
                            nc.tensor.matmul(
                                pso[os_][:], V2[:, tt, os_ * P:(os_ + 1) * P],
                                E[:, tt, :],
                                start=(tt == 0), stop=(tt == NT - 1))
                    ot = opool.tile([P, NE, CH], dt.bfloat16)
                    for os_ in range(NE):
                        nc.vector.tensor_copy(ot[:, os_, :], pso[os_][:])
                        if os_ == 1:
                            nc.gpsimd.dma_start(o4[:, blk, 0:2, :], ot[:, 0:2, :])
                    nc.gpsimd.dma_start(o4[:, blk, 2:NE, :], ot[:, 2:NE, :])

    nc.compile()
    return nc


def _pmajor_feat(x_T):
    """[D, cols] -> [128, NE*cols] with feature plane-major free axis."""
    Dd, cols = x_T.shape
    return np.ascontiguousarray(
        x_T.reshape(NE, P, cols).transpose(1, 0, 2).reshape(P, NE * cols))


def kernel(q, k, v, mask, Wq, bq, Wk, bk, Wv, bv, Wo, bo):
    from concourse.bass_utils import run_bass_kernel_spmd
    import ml_dtypes

    q = np.asarray(q, np.float32)
    k = np.asarray(k, np.float32)
    v = np.asarray(v, np.float32)
    mask = np.asarray(mask)
    Wq = np.asarray(Wq, np.float32)
    Wk = np.asarray(Wk, np.float32)
    Wv = np.asarray(Wv, np.float32)
    Wo = np.asarray(Wo, np.float32)
    bq = np.asarray(bq, np.float32)
    bk = np.asarray(bk, np.float32)
    bv = np.asarray(bv, np.float32)
    bo = np.asarray(bo, np.float32)

    bf16 = ml_dtypes.bfloat16

    # k/v: [128, NE, B, S] flattened; planes of 128 features, batch-major inside
    kT = k.transpose(2, 0, 1).reshape(D, B * S)   # [D, B*S]
    vT = v.transpose(2, 0, 1).reshape(D, B * S)
    kTp = np.ascontiguousarray(
        kT.reshape(NE, P, B * S).transpose(1, 0, 2).reshape(P, NE * B * S)).astype(bf16)
    vTp = np.ascontiguousarray(
        vT.reshape(NE, P, B * S).transpose(1, 0, 2).reshape(P, NE * B * S)).astype(bf16)
    # q: [128, B*NCH, NE, CH] flattened
    qT = q.transpose(2, 0, 1).reshape(D, B, NCH, CH)          # [D, B, NCH, CH]
    qTp = np.ascontiguousarray(
        qT.reshape(NE, P, B, NCH, CH).transpose(1, 2, 3, 0, 4).reshape(P, B * NCH * NE * CH)
    ).astype(bf16)
    # mask: multiplicative {0,1}, [128, NCH, NT, CH] flattened (t on partitions)
    m01 = (mask.T != 1).astype(np.float32)                     # [S(t), S(s)]
    mp = np.ascontiguousarray(
        m01.reshape(NT, P, NCH, CH).transpose(1, 2, 0, 3).reshape(P, NCH * NT * CH)
    ).astype(bf16)

    kf = k.reshape(B * S, D)
    in_maps = []
    for h in range(H):
        Wq64 = Wq[h].astype(np.float64)
        Wk64 = Wk[h].astype(np.float64)
        Wv64 = Wv[h].astype(np.float64)
        Wo64 = Wo[h * D:(h + 1) * D, :].astype(np.float64)
        A = (Wk64 @ Wq64.T * SCALE).astype(np.float32)  # lhsT for K2^T proj
        U = (Wv64 @ Wo64).astype(np.float32)            # rhs for V2 proj
        wvec = (kf @ (Wk[h] @ bq[h])) * SCALE           # per-key exp bias
        wb = np.ascontiguousarray(wvec.reshape(B * NT, P).T.astype(np.float32))
        in_maps.append({
            "qT": qTp, "kT": kTp, "vT": vTp, "mT": mp,
            "ka": _pmajor_feat(A).astype(bf16),
            "vu": _pmajor_feat(U).astype(bf16),
            "wb": wb,
        })

    if "nc" not in _CACHE:
        _CACHE["nc"] = _build()
    nc = _CACHE["nc"]
    _CACHE["in_maps"] = in_maps

    res = run_bass_kernel_spmd(nc, in_maps, core_ids=list(range(H)))
    total = np.zeros((D, B * S), np.float64)
    for h in range(H):
        r = res.results[h]["rs"].sum(axis=0, dtype=np.float64)   # [B*S]
        # undo p-major out layout: [128, B*NCH, NE, CH] -> [D, B*S]
        o = res.results[h]["out"].astype(np.float64)
        o = o.reshape(P, B * NCH, NE, CH).transpose(2, 0, 1, 3).reshape(D, B * S)
        total += o / r[None, :]

    cvec = bo.astype(np.float64).copy()
    for h in range(H):
        cvec += bv[h].astype(np.float64) @ Wo[h * D:(h + 1) * D, :].astype(np.float64)
    total += cvec[:, None]
    return total.T.astype(np.float32).reshape(B, S, D)


# revision 9
# speedup vs baseline: 1.0957x; 1.0113x over previous
"""Multi-head attention TRN2 kernel, head-parallel across 8 NeuronCores.

Per core c (= head h=c), all matmuls in bf16 (full PE rate, half the DMA
and SBUF footprint of f32), keys-on-partitions score layout, with both
outer projections algebraically fused into the K / V projections:

  scores = q Wq (k Wk)^T = q G k^T          G = Wq Wk^T   (host)
  out    = attn (v Wv) Wo = attn (v U)      U = Wv Wo_h   (host)

so the device only computes, per core:

  K2T[e,t] = A k^T   with A = (Wk Wq^T)*scale  (lhsT = A nat, rhs = kT)
  V2[t,o]  = v U                               (lhsT = vT,  rhs = U)
  scoresT[t,s] = K2 q^T                        (lhsT = K2T, rhs = qT chunk)
  E = exp(scoresT) * m'[t,s]                   (ACT exp from PSUM, DVE mask)
  rowsum partials via DVE add tree over E tiles -> rs output
  outT[o,s] = V2^T E                           (lhsT = V2, rhs = E)

where m' = {0,1}-mask * exp(per-key bias from bq), folded on host into a
single fp8 multiplicative mask so the ACT exp needs no bias operand and
can process two 128x512 score tiles per instruction (keeps ACT off the
critical path).

Scheduling: 22 warm-up matmuls on a zeroed tile hold the PE HAM clock
gate at 8/8 while the first inputs stream in; inputs arrive as ONE large
dma_start per (tensor, batch) — per-dma_start queue overhead (~1.5us)
was the real DMA bottleneck — spread across the sync/vector/scalar/
gpsimd queues; all DRAM tensors are partition-major so DMA lines are
4-16 KB contiguous; E / K2T / V2 / mask / q double-buffered; PSUM
evacuation split between ACT and DVE.

Host: bf16-casts and relayouts inputs, folds all biases exactly (bk
drops under softmax; bq -> per-key factor in m'; bv,bo -> final add),
sums per-head partial outputs, divides by the gathered rowsums, and
undoes the partition-major output layout.
"""
import sys
import numpy as np

sys.path.insert(0, "/opt/trn_rl_repo")

H, D, B, S = 8, 512, 2, 2048
P = 128
NE = D // P            # 4 feature tiles
NT = S // P            # 16 key tiles per batch
CH = 512               # query/key chunk width
NCH = S // CH          # 4 chunks per batch
SCALE = 1.0 / np.sqrt(np.float32(D))

_CACHE = {}


def _build():
    from contextlib import ExitStack
    from concourse import bass, bacc, tile

    mybir = bass.mybir
    dt = mybir.dt
    AF = mybir.ActivationFunctionType

    nc = bacc.Bacc("TRN2", target_bir_lowering=False, debug=False)

    # Partition-major DRAM layouts; free axis ordered exactly as consumed.
    qT_d = nc.dram_tensor("qT", [P, B * NCH * NE * CH], dt.bfloat16, kind="ExternalInput")
    kT_d = nc.dram_tensor("kT", [P, B * NE * S], dt.bfloat16, kind="ExternalInput")
    vT_d = nc.dram_tensor("vT", [P, B * NE * S], dt.bfloat16, kind="ExternalInput")
    mT_d = nc.dram_tensor("mT", [P, B * NCH * NT * CH], dt.float8e4, kind="ExternalInput")
    ka_d = nc.dram_tensor("ka", [P, NE * D], dt.bfloat16, kind="ExternalInput")  # (Wk Wq^T)*scale
    vu_d = nc.dram_tensor("vu", [P, NE * D], dt.bfloat16, kind="ExternalInput")  # Wv Wo_h
    out_d = nc.dram_tensor("out", [P, B * NCH * NE * CH], dt.bfloat16, kind="ExternalOutput")
    rs_d = nc.dram_tensor("rs", [P, B * S], dt.float32, kind="ExternalOutput")

    q3 = qT_d.ap().rearrange("p (b r) -> p b r", b=B)               # [128, B, NCH*NE*CH]
    k3 = kT_d.ap().rearrange("p (b r) -> p b r", b=B)               # [128, B, NE*S]
    v3 = vT_d.ap().rearrange("p (b r) -> p b r", b=B)
    m5 = mT_d.ap().rearrange("p (b c a s) -> p b c a s", b=B, c=NCH, a=NT)
    o4 = out_d.ap().rearrange("p (k a c) -> p k a c", a=NE, c=CH)   # [128, B*NCH, NE, CH]

    with tile.TileContext(nc) as tc:
        with ExitStack() as ctx:
            wpool = ctx.enter_context(tc.tile_pool(name="w", bufs=1))
            kvpool = ctx.enter_context(tc.tile_pool(name="kv", bufs=2))
            xin = ctx.enter_context(tc.tile_pool(name="xin", bufs=2))
            qpool = ctx.enter_context(tc.tile_pool(name="q", bufs=2))
            epool = ctx.enter_context(tc.tile_pool(name="e", bufs=2))
            mpool = ctx.enter_context(tc.tile_pool(name="m", bufs=2))
            rpool = ctx.enter_context(tc.tile_pool(name="r", bufs=1))
            opool = ctx.enter_context(tc.tile_pool(name="o", bufs=1))
            psA = ctx.enter_context(tc.tile_pool(name="psA", bufs=2, space="PSUM"))
            psO = ctx.enter_context(tc.tile_pool(name="psO", bufs=4, space="PSUM"))

            # --- PE warm-up: matmuls on a zeroed tile keep the HAM clock
            # gate at 8/8 until the first k bytes land (~12us). ---
            warm = wpool.tile([P, CH], dt.bfloat16)
            nc.vector.memset(warm[:], 0.0)
            for i in range(22):
                pw = psO.tile([P, CH], dt.float32, tag="pso")
                nc.tensor.matmul(pw[:], warm[:, 0:P], warm[:], start=True, stop=True)

            ka = wpool.tile([P, NE, D], dt.bfloat16)
            vu = wpool.tile([P, NE, D], dt.bfloat16)

            # One large dma_start per (tensor, batch), spread over 4 queues:
            # sync carries only the critical k(b0) first.
            kins, vins, qins = {}, {}, {}
            for b in range(B):
                kins[b] = xin.tile([P, NE, S], dt.bfloat16, tag="xin", name=f"kin{b}")
                nc.sync.dma_start(kins[b][:], k3[:, b, :].rearrange("p (a t) -> p a t", a=NE))
            nc.scalar.dma_start(ka[:], ka_d.ap().rearrange("p (a e) -> p a e", a=NE))
            nc.scalar.dma_start(vu[:], vu_d.ap().rearrange("p (a e) -> p a e", a=NE))
            for b in range(B):
                vins[b] = xin.tile([P, NE, S], dt.bfloat16, tag="xin", name=f"vin{b}")
                nc.gpsimd.dma_start(vins[b][:], v3[:, b, :].rearrange("p (a t) -> p a t", a=NE))
                qins[b] = qpool.tile([P, NCH, NE, CH], dt.bfloat16, tag="qin", name=f"qin{b}")
                nc.scalar.dma_start(
                    qins[b][:], q3[:, b, :].rearrange("p (c a s) -> p c a s", c=NCH, a=NE))

            # ---- stage A: K2^T (scores-critical) then V2, both batches ----
            K2Ts, V2s = {}, {}
            for b in range(B):
                K2Ts[b] = kvpool.tile([P, NE, S], dt.bfloat16, tag="K2T", name=f"K2T{b}")
                kin = kins[b]
                for tc4 in range(NCH):
                    for g in range(NE // 2):
                        ps = psA.tile([P, 2, CH], dt.float32, tag="ps")
                        for hf in range(2):
                            et = 2 * g + hf
                            for kd in range(NE):
                                nc.tensor.matmul(
                                    ps[:, hf, :], ka[:, kd, et * P:(et + 1) * P],
                                    kin[:, kd, tc4 * CH:(tc4 + 1) * CH],
                                    start=(kd == 0), stop=(kd == NE - 1))
                        nc.scalar.copy(
                            K2Ts[b][:, 2 * g:2 * g + 2, tc4 * CH:(tc4 + 1) * CH], ps[:])
            for b in range(B):
                V2s[b] = kvpool.tile([P, NT, D], dt.bfloat16, tag="V2", name=f"V2{b}")
                vin = vins[b]
                for tc4 in range(NCH):
                    for g in range(2):
                        ps = psA.tile([P, 2, CH], dt.float32, tag="ps")
                        for hf in range(2):
                            tl = tc4 * 4 + 2 * g + hf
                            for kd in range(NE):
                                nc.tensor.matmul(
                                    ps[:, hf, :], vin[:, kd, tl * P:(tl + 1) * P],
                                    vu[:, kd, :],
                                    start=(kd == 0), stop=(kd == NE - 1))
                        nc.vector.tensor_copy(V2s[b][:, tc4 * 4 + 2 * g:tc4 * 4 + 2 * g + 2, :], ps[:])

            # ---- stage B: per (chunk, batch) attention; mask loaded once per chunk ----
            for c in range(NCH):
                for b in range(B):
                    mt = mpool.tile([P, NT, CH], dt.float8e4)
                    nc.gpsimd.dma_start(mt[:], m5[:, b, c, :, :])
                    blk = b * NCH + c
                    col0 = b * S + c * CH
                    K2T, V2 = K2Ts[b], V2s[b]
                    qin = qins[b]
                    last = (c == NCH - 1) and (b == B - 1)

                    E = epool.tile([P, NT, CH], dt.bfloat16)
                    for g in range(NT // 2):
                        ps = psA.tile([P, 2, CH], dt.float32, tag="ps")
                        for hf in range(2):
                            tt = 2 * g + hf
                            for et in range(NE):
                                nc.tensor.matmul(
                                    ps[:, hf, :], K2T[:, et, tt * P:(tt + 1) * P],
                                    qin[:, c, et, :],
                                    start=(et == 0), stop=(et == NE - 1))
                        nc.scalar.activation(E[:, 2 * g:2 * g + 2, :], ps[:], AF.Exp)
                        nc.vector.tensor_mul(
                            E[:, 2 * g:2 * g + 2, :], E[:, 2 * g:2 * g + 2, :],
                            mt[:, 2 * g:2 * g + 2, :])

                    red = rpool.tile([P, NT // 2, CH], dt.float32, tag="red")
                    nc.vector.tensor_add(red[:], E[:, 0:8, :], E[:, 8:16, :])
                    nc.vector.tensor_add(red[:, 0:4, :], red[:, 0:4, :], red[:, 4:8, :])
                    nc.vector.tensor_add(red[:, 0:2, :], red[:, 0:2, :], red[:, 2:4, :])
                    accr = rpool.tile([P, CH], dt.float32, tag="accr")
                    nc.vector.tensor_add(accr[:], red[:, 0, :], red[:, 1, :])
                    nc.gpsimd.dma_start(rs_d[:, col0:col0 + CH], accr[:])

                    pso = [psO.tile([P, CH], dt.float32, tag="pso", name=f"pso{i}") for i in range(NE)]
                    for tt in range(NT):
                        for os_ in range(NE):
                            nc.tensor.matmul(
                                pso[os_][:], V2[:, tt, os_ * P:(os_ + 1) * P],
                                E[:, tt, :],
                                start=(tt == 0), stop=(tt == NT - 1))
                    ot = opool.tile([P, NE, CH], dt.bfloat16)
                    nc.scalar.copy(ot[:, 0, :], pso[0][:])
                    nc.vector.tensor_copy(ot[:, 2, :], pso[2][:])
                    nc.scalar.copy(ot[:, 1, :], pso[1][:])
                    nc.vector.tensor_copy(ot[:, 3, :], pso[3][:])
                    nc.gpsimd.dma_start(o4[:, blk, 0:2, :], ot[:, 0:2, :])
                    if last:
                        nc.sync.dma_start(o4[:, blk, 2:NE, :], ot[:, 2:NE, :])
                    else:
                        nc.gpsimd.dma_start(o4[:, blk, 2:NE, :], ot[:, 2:NE, :])

    nc.compile()
    return nc


def _pmajor_feat(x_T):
    """[D, cols] -> [128, NE*cols] with feature plane-major free axis."""
    Dd, cols = x_T.shape
    return np.ascontiguousarray(
        x_T.reshape(NE, P, cols).transpose(1, 0, 2).reshape(P, NE * cols))


def kernel(q, k, v, mask, Wq, bq, Wk, bk, Wv, bv, Wo, bo):
    from concourse.bass_utils import run_bass_kernel_spmd
    import ml_dtypes

    q = np.asarray(q, np.float32)
    k = np.asarray(k, np.float32)
    v = np.asarray(v, np.float32)
    mask = np.asarray(mask)
    Wq = np.asarray(Wq, np.float32)
    Wk = np.asarray(Wk, np.float32)
    Wv = np.asarray(Wv, np.float32)
    Wo = np.asarray(Wo, np.float32)
    bq = np.asarray(bq, np.float32)
    bk = np.asarray(bk, np.float32)
    bv = np.asarray(bv, np.float32)
    bo = np.asarray(bo, np.float32)

    bf16 = ml_dtypes.bfloat16
    f8 = ml_dtypes.float8_e4m3fn

    # k/v: [128, B, NE, S] flattened (batch-major so one DMA per batch
    # reads a 16KB contiguous run per partition)
    kT = k.transpose(2, 0, 1).reshape(D, B * S)   # [D, B*S]
    vT = v.transpose(2, 0, 1).reshape(D, B * S)
    kTp = np.ascontiguousarray(
        kT.reshape(NE, P, B, S).transpose(1, 2, 0, 3).reshape(P, B * NE * S)).astype(bf16)
    vTp = np.ascontiguousarray(
        vT.reshape(NE, P, B, S).transpose(1, 2, 0, 3).reshape(P, B * NE * S)).astype(bf16)
    # q: [128, B, NCH, NE, CH] flattened
    qT = q.transpose(2, 0, 1).reshape(D, B, NCH, CH)
    qTp = np.ascontiguousarray(
        qT.reshape(NE, P, B, NCH, CH).transpose(1, 2, 3, 0, 4).reshape(P, B * NCH * NE * CH)
    ).astype(bf16)
    # multiplicative mask {0,1}, [128, NCH, NT, CH] (t on partitions)
    m01 = (mask.T != 1).astype(np.float32)                     # [S(t), S(s)]

    kf = k.reshape(B * S, D)
    in_maps = []
    for h in range(H):
        Wq64 = Wq[h].astype(np.float64)
        Wk64 = Wk[h].astype(np.float64)
        Wv64 = Wv[h].astype(np.float64)
        Wo64 = Wo[h * D:(h + 1) * D, :].astype(np.float64)
        A = (Wk64 @ Wq64.T * SCALE).astype(np.float32)  # lhsT for K2^T proj
        U = (Wv64 @ Wo64).astype(np.float32)            # rhs for V2 proj
        # fold bq into the mask as a per-(batch,key) multiplicative
        # factor exp(k Wk bq * scale) -- identical to an additive exp bias.
        wvec = (kf @ (Wk[h] @ bq[h])) * SCALE           # [B*S] per-key bias
        mh = m01[None, :, :] * np.exp(wvec).reshape(B, S)[:, :, None]  # [B, S(t), S(s)]
        mp = np.ascontiguousarray(
            mh.reshape(B, NT, P, NCH, CH).transpose(2, 0, 3, 1, 4).reshape(P, B * NCH * NT * CH)
        ).astype(f8)
        in_maps.append({
            "qT": qTp, "kT": kTp, "vT": vTp, "mT": mp,
            "ka": _pmajor_feat(A).astype(bf16),
            "vu": _pmajor_feat(U).astype(bf16),
        })

    if "nc" not in _CACHE:
        _CACHE["nc"] = _build()
    nc = _CACHE["nc"]
    _CACHE["in_maps"] = in_maps

    res = run_bass_kernel_spmd(nc, in_maps, core_ids=list(range(H)))
    total = np.zeros((D, B * S), np.float64)
    for h in range(H):
        r = res.results[h]["rs"].sum(axis=0, dtype=np.float64)   # [B*S]
        o = res.results[h]["out"].astype(np.float64)
        o = o.reshape(P, B * NCH, NE, CH).transpose(2, 0, 1, 3).reshape(D, B * S)
        total += o / r[None, :]

    cvec = bo.astype(np.float64).copy()
    for h in range(H):
        cvec += bv[h].astype(np.float64) @ Wo[h * D:(h + 1) * D, :].astype(np.float64)
    total += cvec[:, None]
    return total.T.astype(np.float32).reshape(B, S, D)
